# revision 1
# baseline (speedup 1.0000x reference)
"""2-layer GCN encoder on 8 Trainium2 NeuronCores (Bass/Tile).

Math: out = relu(Dinv (A+I) Dinv (x W) + b) twice, Dinv = deg^-1/2.
Factored as: table = (dinv * x) @ W ; agg[v] = sum_{e: dst=v} table[src_e] ;
out[v] = relu(dinv[v] * agg[v] + b)   -- no per-edge weights needed.

Distribution: dst-node sharding. Node ids padded to 100352 = 784 windows of
128. Core p owns 98 windows. Each core builds the FULL table locally from the
(replicated) layer input, then gathers + segment-sums only the edges that
point into its own windows. The inter-layer "halo exchange" (all-gather of
layer-1 activations) happens on the host between the two SPMD invocations of
the same compiled program.

Gather indices are int16 (reach 32768), so sources are split into 4 blocks
with per-block base offsets on the gather's table AP. Per (window, block) the
edge count is data-dependent while gather calls need static shapes, so the
host computes per-block caps (128-aligned) from the actual graph and pads
with repeats of block-row 0. Padded slots carry lid = -1 so their one-hot
column in S is all-zero and they contribute nothing.

Slot layout per batch of B windows (block-major so each gather call's slots
are contiguous): [blk0: w0 cap0, w1 cap0 | blk1: w0 cap1, w1 cap1 | ...].
Segment-sum on the tensor engine: per 128-slot tile, S[e, j] = (lid[e] == j)
built by the vector engine, then psum[dst, feat] += S.T @ msgs accumulated
over the window's tiles.
"""
import sys
sys.path.insert(0, "/opt/trn_rl_repo")

import math
import os
import numpy as np

N = 100000
F = 128
NCORES = 8
WIN = 128                      # dst nodes per window
NPAD = 100352                  # 784 * 128
NW = NPAD // WIN               # 784 windows
WPC = NW // NCORES             # 98 windows per core
BLOCK = 32768                  # gather idx block (int16 reach)
NBLK = 4                       # 3*32768 + 2048 = 100352
B = 2                          # windows per gather batch
NB = WPC // B                  # 49 batches

_compiled = None               # (nc, cfg) cache across invocations
_last_exec_ns = None           # filled when KERNEL_TRACE=1
_last_wall_s = None            # wall time of device calls (incl transfers)


def _wrap_idx(flat):
    """[n] -> [128, n/16] int16: slot i -> (i%16, i//16), replicated x8."""
    n = len(flat)
    m = np.asarray(flat, np.int16).reshape(n // 16, 16).T
    return np.tile(m, (8, 1))


def _host_prep(edge_index):
    """Shard edges, build per-core gather indices / lids / caps."""
    src = np.concatenate([edge_index[0], np.arange(N, dtype=np.int64)])
    dst = np.concatenate([edge_index[1], np.arange(N, dtype=np.int64)])
    deg = np.bincount(dst, minlength=NPAD).astype(np.float32)
    deg[N:] = 1.0

    g = (src // BLOCK).astype(np.int64)           # src block 0..3
    w = (dst // WIN).astype(np.int64)             # global window 0..783
    order = np.lexsort((src, g, w))               # by (window, block, src)
    src, dst, g, w = src[order], dst[order], g[order], w[order]
    lid = (dst % WIN).astype(np.float32)
    loc = src - g * BLOCK                         # in-block idx (< 32768)

    counts = np.zeros((NW, NBLK), np.int64)
    np.add.at(counts, (w, g), 1)
    caps = [int(128 * math.ceil(max(int(counts[:, b].max()), 1) / 128))
            for b in range(NBLK)]
    tw = sum(caps) // 128                         # tiles per window
    cum = np.concatenate([[0], np.cumsum(counts.reshape(-1))])  # run starts

    idxs = [np.zeros((NCORES, NB, 128, (B * caps[b]) // 16), np.int16)
            for b in range(NBLK)]
    lids = np.full((NCORES, NB, 128, B * tw), -1.0, np.float32)
    btb = np.concatenate([[0], np.cumsum([c // 128 for c in caps])])

    for c in range(NCORES):
        for b in range(NB):
            for blk in range(NBLK):
                cap = caps[blk]
                stream = np.zeros(B * cap, np.int64)
                lstream = np.full(B * cap, -1.0, np.float32)
                for r in range(B):
                    wg = (c * WPC + b * B + r) * NBLK + blk
                    s0, s1 = cum[wg], cum[wg + 1]
                    nn = s1 - s0
                    stream[r * cap : r * cap + nn] = loc[s0:s1]
                    lstream[r * cap : r * cap + nn] = lid[s0:s1]
                idxs[blk][c, b] = _wrap_idx(stream)
                # batch tile grid: block region starts at tile B*btb[blk];
                # window r owns cap/128 tiles within it
                seg = lstream.reshape(B * cap // 128, 128)
                t0 = B * btb[blk]
                lids[c, b, :, t0 : t0 + B * cap // 128] = seg.T
    cfg = {"caps": tuple(caps), "tw": int(tw),
           "btb": tuple(int(x) for x in btb)}
    data = {"idxs": idxs, "lids": lids, "degT": deg.reshape(NW, 128).T.copy()}
    return cfg, data


def _win_tiles(cfg, r):
    """Tile indices (within a batch's tile grid) owned by window r."""
    caps, btb = cfg["caps"], cfg["btb"]
    tiles = []
    for blk in range(NBLK):
        cb = caps[blk] // 128
        base = B * btb[blk] + r * cb
        tiles.extend(range(base, base + cb))
    return tiles


def _build_nc(cfg):
    from concourse import bacc, mybir
    import concourse.tile as tile
    from concourse import library_config
    import contextlib

    dt = mybir.dt
    caps, tw, btb = cfg["caps"], cfg["tw"], cfg["btb"]
    bases = [0, BLOCK, 2 * BLOCK, 3 * BLOCK]
    sizes = [BLOCK, BLOCK, BLOCK, NPAD - 3 * BLOCK]

    nc = bacc.Bacc("TRN2", target_bir_lowering=False, debug=False,
                   num_devices=NCORES)
    feat = nc.dram_tensor("feat", [NPAD, F], dt.float32, kind="ExternalInput")
    wmat = nc.dram_tensor("wmat", [F, F], dt.float32, kind="ExternalInput")
    btile = nc.dram_tensor("btile", [128, F], dt.float32, kind="ExternalInput")
    iota = nc.dram_tensor("iota", [128, 128], dt.float32, kind="ExternalInput")
    ident = nc.dram_tensor("ident", [128, 128], dt.float32, kind="ExternalInput")
    degT = nc.dram_tensor("degT", [128, NW], dt.float32, kind="ExternalInput")
    degw = nc.dram_tensor("degw", [128, WPC], dt.float32, kind="ExternalInput")
    idxt = [
        nc.dram_tensor(f"idx{b}", [NB, 128, (B * caps[b]) // 16], dt.int16,
                       kind="ExternalInput")
        for b in range(NBLK)
    ]
    lidt = nc.dram_tensor("lids", [NB, 128, B * tw], dt.float32,
                          kind="ExternalInput")
    table = nc.dram_tensor("table", [NPAD, F], dt.float32, kind="Internal")
    out = nc.dram_tensor("out", [WPC * WIN, F], dt.float32,
                         kind="ExternalOutput")

    with tile.TileContext(nc) as tc:
        ctx = contextlib.ExitStack()
        with ctx:
            cpool = ctx.enter_context(tc.tile_pool(name="const", bufs=1))
            bpool = ctx.enter_context(tc.tile_pool(name="build", bufs=3))
            mpool = ctx.enter_context(tc.tile_pool(name="msg", bufs=2))
            spool = ctx.enter_context(tc.tile_pool(name="sprep", bufs=6))
            epool = ctx.enter_context(tc.tile_pool(name="epi", bufs=3))
            pps = ctx.enter_context(tc.tile_pool(name="ps", bufs=2, space="PSUM"))

            nc.gpsimd.load_library(library_config.mlp)

            # ---- constants
            t_iota = cpool.tile([128, 128], dt.float32, tag="iota")
            nc.sync.dma_start(t_iota[:], iota.ap()[:, :])
            t_id = cpool.tile([128, 128], dt.float32, tag="ident")
            nc.sync.dma_start(t_id[:], ident.ap()[:, :])
            t_w = cpool.tile([F, F], dt.float32, tag="w")
            nc.sync.dma_start(t_w[:], wmat.ap()[:, :])
            t_b = cpool.tile([128, F], dt.float32, tag="b")
            nc.sync.dma_start(t_b[:], btile.ap()[:, :])

            t_degT = cpool.tile([128, NW], dt.float32, tag="degT")
            nc.sync.dma_start(t_degT[:], degT.ap()[:, :])
            t_dinv = cpool.tile([128, NW], dt.float32, tag="dinv")
            nc.vector.reciprocal(t_dinv[:], t_degT[:])
            nc.scalar.activation(t_dinv[:], t_dinv[:],
                                 mybir.ActivationFunctionType.Sqrt)
            t_degw = cpool.tile([128, WPC], dt.float32, tag="degw")
            nc.sync.dma_start(t_degw[:], degw.ap()[:, :])
            t_dinw = cpool.tile([128, WPC], dt.float32, tag="dinw")
            nc.vector.reciprocal(t_dinw[:], t_degw[:])
            nc.scalar.activation(t_dinw[:], t_dinw[:],
                                 mybir.ActivationFunctionType.Sqrt)

            # ---- build full table: h = (dinv * feat) @ W
            for bt in range(int(os.environ.get("KN_NWB", NW))):
                t_x = bpool.tile([128, F], dt.float32, tag="x")
                nc.sync.dma_start(t_x[:], feat.ap()[bt * 128 : (bt + 1) * 128, :])
                t_xs = bpool.tile([128, F], dt.float32, tag="xs")
                nc.vector.tensor_scalar(
                    t_xs[:], t_x[:], t_dinv[:, bt : bt + 1], None,
                    mybir.AluOpType.mult,
                )
                p_xT = pps.tile([128, 128], dt.float32, tag="xT")
                nc.tensor.transpose(p_xT[:], t_xs[:], t_id[:])
                t_xsT = bpool.tile([128, F], dt.float32, tag="xsT")
                nc.vector.tensor_copy(t_xsT[:], p_xT[:])
                p_h = pps.tile([128, F], dt.float32, tag="h")
                nc.tensor.matmul(p_h[:], t_xsT[:], t_w[:], start=True, stop=True)
                t_h = bpool.tile([128, F], dt.float32, tag="h")
                nc.vector.tensor_copy(t_h[:], p_h[:])
                nc.sync.dma_start(table.ap()[bt * 128 : (bt + 1) * 128, :], t_h[:])

            # table complete before any gather reads it
            tc.strict_bb_all_engine_barrier()

            # ---- gather + aggregate per batch of B windows
            for b in range(int(os.environ.get("KN_NB", NB))):
                t_msg = mpool.tile([128, B * tw, F], dt.float32, tag="msg")
                t_lid = spool.tile([128, B * tw], dt.float32, tag="lid")
                nc.sync.dma_start(t_lid[:], lidt.ap()[b, :, :])
                for blk in range(NBLK):
                    cap = caps[blk]
                    t_ix = spool.tile([128, (B * cap) // 16], dt.int16,
                                      tag=f"ix{blk}")
                    nc.sync.dma_start(t_ix[:], idxt[blk].ap()[b, :, :])
                    t0 = B * btb[blk]
                    nc.gpsimd.dma_gather(
                        t_msg[:, t0 : t0 + (B * cap) // 128, :],
                        table.ap()[bases[blk] : bases[blk] + sizes[blk], :],
                        t_ix[:],
                        B * cap, B * cap, F,
                        single_packet=False,
                    )
                for r in range(B):
                    k = b * B + r          # window index within core
                    p_agg = pps.tile([128, F], dt.float32, tag="agg")
                    wt = _win_tiles(cfg, r)
                    for j, t in enumerate(wt):
                        t_S = spool.tile([128, 128], dt.float32, tag="S")
                        nc.vector.tensor_scalar(
                            t_S[:], t_iota[:], t_lid[:, t : t + 1], None,
                            mybir.AluOpType.is_equal,
                        )
                        nc.tensor.matmul(
                            p_agg[:], t_S[:], t_msg[:, t, :],
                            start=(j == 0), stop=(j == len(wt) - 1),
                        )
                    t_e = epool.tile([128, F], dt.float32, tag="e")
                    nc.vector.tensor_scalar(
                        t_e[:], p_agg[:], t_dinw[:, k : k + 1], None,
                        mybir.AluOpType.mult,
                    )
                    nc.vector.tensor_tensor(
                        t_e[:], t_e[:], t_b[:], mybir.AluOpType.add
                    )
                    t_o = epool.tile([128, F], dt.float32, tag="o")
                    nc.scalar.activation(
                        t_o[:], t_e[:], mybir.ActivationFunctionType.Relu
                    )
                    nc.sync.dma_start(
                        out.ap()[k * 128 : (k + 1) * 128, :], t_o[:]
                    )

    nc.compile()
    return nc


def _run_layer(nc, data, feat_pad, W, bias):
    from concourse.bass_utils import run_bass_kernel_spmd

    iota = np.tile(np.arange(128, dtype=np.float32)[None, :], (128, 1))
    ident = np.eye(128, dtype=np.float32)
    btile = np.tile(np.asarray(bias, np.float32)[None, :], (128, 1))
    in_maps = []
    for c in range(NCORES):
        m = {
            "feat": feat_pad,
            "wmat": np.asarray(W, np.float32),
            "btile": btile,
            "iota": iota,
            "ident": ident,
            "degT": data["degT"],
            "degw": data["degT"][:, c * WPC : (c + 1) * WPC].copy(),
            "lids": data["lids"][c],
        }
        for b in range(NBLK):
            m[f"idx{b}"] = data["idxs"][b][c]
        in_maps.append(m)
    import time as _time
    trace = False  # NTFF hook unavailable in this container
    t0 = _time.time()
    res = run_bass_kernel_spmd(nc, in_maps, core_ids=list(range(NCORES)),
                               trace=trace)
    global _last_wall_s, _last_exec_ns
    _last_wall_s = (_last_wall_s or 0.0) + (_time.time() - t0)
    if trace:
        ns = getattr(res, "exec_time_ns", None)
        if ns:
            _last_exec_ns = (_last_exec_ns or 0) + ns
    return np.concatenate([res.results[c]["out"] for c in range(NCORES)], axis=0)


def kernel(x, edge_index, W1, b1, W2, b2):
    global _compiled
    x = np.asarray(x, np.float32)
    edge_index = np.asarray(edge_index)
    cfg, data = _host_prep(edge_index)
    if _compiled is None or _compiled[1] != cfg:
        _compiled = (_build_nc(cfg), cfg)
    nc = _compiled[0]

    xpad = np.zeros((NPAD, F), np.float32)
    xpad[:N] = x
    out1 = _run_layer(nc, data, xpad, W1, b1)        # [NPAD, F] relu'd
    h1 = np.zeros((NPAD, F), np.float32)
    h1[:N] = out1[:N]
    out2 = _run_layer(nc, data, h1, W2, b2)
    return out2[:N].astype(np.float32)



# revision 4
# speedup vs baseline: 7.2530x; 7.2530x over previous
"""2-layer GCN encoder on 8 Trainium2 NeuronCores (Bass/Tile), single-shot.

Math: out = relu(Dinv (A+I) Dinv (x W) + b) twice, Dinv = deg^-1/2.
Factored as: table = (dinv * x) @ W ; agg[v] = sum_{e: dst=v} table[src_e] ;
out[v] = relu(dinv[v] * agg[v] + b)   -- no per-edge weights needed.

Distribution: dst-node sharding, one device invocation for BOTH layers.
Node ids padded to 100352 = 784 windows of 128; core p owns 98 windows
(12544 rows). Each core receives only its own x rows (fp16), builds its
table shard (dinv*x)@W1, and the full table is assembled on-device with
an 8-core AllGather over NeuronLink. Layer-1 aggregation fuses the
layer-2 shard build ((dinv*relu(...))@W2) in its epilogue, a second
AllGather publishes it, and layer-2 aggregation writes the fp16 output
shard. Host traffic is ~65MB total vs ~930MB for the two-invocation
replicated-table variant; the inter-layer halo exchange never leaves the
device.

Gather indices are int16 (reach 32768), so sources are split into 4
blocks with per-block base offsets on the gather's table AP. Per
(window, block) the edge count is data-dependent while gather calls need
static shapes, so the host computes per-block caps (128-aligned) from
the actual graph and pads with repeats of block-row 0. Padded slots
carry lid = -1 so their one-hot column in S is all-zero and they
contribute nothing. Indices are shipped packed in 16 partitions and
replicated to the gather's 128-partition layout on-device (8 DRAM->DRAM
copies), cutting their host transfer 8x.

Slot layout per batch of B windows (block-major so each gather call's
slots are contiguous): [blk0: w0 cap0, w1 cap0 | blk1: w0 cap1, ...].
Segment-sum on the tensor engine: per 128-slot tile, S[e, j] =
(lid[e] == j) built by the vector engine, then psum[dst, feat] +=
S.T @ msgs accumulated over the window's tiles.
"""
import sys
sys.path.insert(0, "/opt/trn_rl_repo")

import math
import time
import numpy as np

N = 100000
F = 128
NCORES = 8
WIN = 128                      # dst nodes per window
NPAD = 100352                  # 784 * 128
NW = NPAD // WIN               # 784 windows
WPC = NW // NCORES             # 98 windows per core
SH = WPC * WIN                 # 12544 rows per core
BLOCK = 32768                  # gather idx block (int16 reach)
NBLK = 4                       # 3*32768 + 2048 = 100352
B = 2                          # windows per gather batch
NB = WPC // B                  # 49 batches

_compiled = None               # (nc, cfg) cache across invocations
_last_exec_ns = None           # filled when a real trace is available
_last_wall_s = None            # wall time of device calls (incl transfers)


def _host_prep(edge_index):
    """Build per-core gather indices / lids / caps (fully vectorized)."""
    e0 = np.asarray(edge_index[0], np.int64)
    e1 = np.asarray(edge_index[1], np.int64)
    loop = np.arange(N, dtype=np.int64)
    src = np.concatenate([e0, loop])
    dst = np.concatenate([e1, loop])
    deg = np.bincount(dst, minlength=NPAD).astype(np.float32)
    deg[N:] = 1.0

    g = src >> 15                                 # src block 0..3
    w = dst >> 7                                  # global window 0..783
    order = np.lexsort((src, g, w))               # by (window, block, src)
    src, dst, g, w = src[order], dst[order], g[order], w[order]

    grp = w * NBLK + g
    counts = np.bincount(grp, minlength=NW * NBLK).reshape(NW, NBLK)
    caps = [int(128 * math.ceil(max(int(counts[:, blk].max()), 1) / 128))
            for blk in range(NBLK)]
    tw = sum(caps) // 128                         # tiles per window
    btb = [0]
    for cap in caps:
        btb.append(btb[-1] + cap // 128)
    cum = np.concatenate([[0], np.cumsum(counts.reshape(-1))])

    j = np.arange(len(src)) - cum[grp]            # rank within (w, blk) run
    c = w // WPC                                  # owning core
    k = w % WPC                                   # window within core
    b = k // B                                    # gather batch
    r = k % B                                     # window within batch

    idxs = []
    lids_flat = np.full(NCORES * NB * 128 * (B * tw), -1.0, np.float16)
    lidv = (dst & 127).astype(np.float16)
    for blk in range(NBLK):
        m = g == blk
        cap = caps[blk]
        flat = np.zeros(NCORES * NB * B * cap, np.int64)
        addr = ((c[m] * NB + b[m]) * B + r[m]) * cap + j[m]
        flat[addr] = src[m] - blk * BLOCK         # in-block idx (< 32768)
        # [n] slot stream -> [16, n/16]: slot i -> (i%16, i//16)
        idxs.append(flat.reshape(NCORES, NB, (B * cap) // 16, 16)
                    .transpose(0, 1, 3, 2).astype(np.int16).copy())
        pos = r[m] * cap + j[m]                   # slot within batch stream
        laddr = (((c[m] * NB + b[m]) * 128 + (pos & 127)) * (B * tw)
                 + B * btb[blk] + (pos >> 7))
        lids_flat[laddr] = lidv[m]
    lids = lids_flat.reshape(NCORES, NB, 128, B * tw)

    cfg = {"caps": tuple(caps), "tw": int(tw), "btb": tuple(btb)}
    data = {"idxs": idxs, "lids": lids,
            "degT": deg.reshape(NW, 128).T.copy()}
    return cfg, data


def _win_tiles(cfg, r):
    """Tile indices (within a batch's tile grid) owned by window r."""
    caps, btb = cfg["caps"], cfg["btb"]
    tiles = []
    for blk in range(NBLK):
        cb = caps[blk] // 128
        base = B * btb[blk] + r * cb
        tiles.extend(range(base, base + cb))
    return tiles


def _build_nc(cfg):
    from concourse import bacc, mybir
    import concourse.tile as tile
    from concourse import library_config
    import contextlib

    dt = mybir.dt
    caps, tw, btb = cfg["caps"], cfg["tw"], cfg["btb"]
    bases = [0, BLOCK, 2 * BLOCK, 3 * BLOCK]
    sizes = [BLOCK, BLOCK, BLOCK, NPAD - 3 * BLOCK]

    nc = bacc.Bacc("TRN2", target_bir_lowering=False, debug=False,
                   num_devices=NCORES)
    xs16 = nc.dram_tensor("xs16", [SH, F], dt.float16, kind="ExternalInput")
    wmat1 = nc.dram_tensor("wmat1", [F, F], dt.float32, kind="ExternalInput")
    wmat2 = nc.dram_tensor("wmat2", [F, F], dt.float32, kind="ExternalInput")
    btile1 = nc.dram_tensor("btile1", [128, F], dt.float32, kind="ExternalInput")
    btile2 = nc.dram_tensor("btile2", [128, F], dt.float32, kind="ExternalInput")
    iota = nc.dram_tensor("iota", [128, 128], dt.float32, kind="ExternalInput")
    ident = nc.dram_tensor("ident", [128, 128], dt.float32, kind="ExternalInput")
    degw = nc.dram_tensor("degw", [128, WPC], dt.float32, kind="ExternalInput")
    idxt = [
        nc.dram_tensor(f"idx{blk}", [NB, 16, (B * caps[blk]) // 16], dt.int16,
                       kind="ExternalInput")
        for blk in range(NBLK)
    ]
    lidt = nc.dram_tensor("lids", [NB, 128, B * tw], dt.float16,
                          kind="ExternalInput")
    idxr = [
        nc.dram_tensor(f"idxr{blk}", [NB, 128, (B * caps[blk]) // 16],
                       dt.int16, kind="Internal")
        for blk in range(NBLK)
    ]
    shard1 = nc.dram_tensor("shard1", [SH, F], dt.float32, kind="Internal")
    shard2 = nc.dram_tensor("shard2", [SH, F], dt.float32, kind="Internal")
    table = nc.dram_tensor("table", [NPAD, F], dt.float32, kind="Internal",
                           addr_space="Shared")
    out16 = nc.dram_tensor("out16", [SH, F], dt.float16,
                           kind="ExternalOutput")

    groups = [list(range(NCORES))]

    with tile.TileContext(nc) as tc:
        ctx = contextlib.ExitStack()
        with ctx:
            cpool = ctx.enter_context(tc.tile_pool(name="const", bufs=1))
            bpool = ctx.enter_context(tc.tile_pool(name="build", bufs=3))
            mpool = ctx.enter_context(tc.tile_pool(name="msg", bufs=2))
            spool = ctx.enter_context(tc.tile_pool(name="sprep", bufs=6))
            epool = ctx.enter_context(tc.tile_pool(name="epi", bufs=3))
            pps = ctx.enter_context(tc.tile_pool(name="ps", bufs=2, space="PSUM"))

            nc.gpsimd.load_library(library_config.mlp)

            # ---- constants
            t_iota = cpool.tile([128, 128], dt.float32, tag="iota")
            nc.sync.dma_start(t_iota[:], iota.ap()[:, :])
            t_id = cpool.tile([128, 128], dt.float32, tag="ident")
            nc.sync.dma_start(t_id[:], ident.ap()[:, :])
            t_w1 = cpool.tile([F, F], dt.float32, tag="w1")
            nc.sync.dma_start(t_w1[:], wmat1.ap()[:, :])
            t_w2 = cpool.tile([F, F], dt.float32, tag="w2")
            nc.sync.dma_start(t_w2[:], wmat2.ap()[:, :])
            t_b1 = cpool.tile([128, F], dt.float32, tag="b1")
            nc.sync.dma_start(t_b1[:], btile1.ap()[:, :])
            t_b2 = cpool.tile([128, F], dt.float32, tag="b2")
            nc.sync.dma_start(t_b2[:], btile2.ap()[:, :])

            t_degw = cpool.tile([128, WPC], dt.float32, tag="degw")
            nc.sync.dma_start(t_degw[:], degw.ap()[:, :])
            t_dinw = cpool.tile([128, WPC], dt.float32, tag="dinw")
            nc.vector.reciprocal(t_dinw[:], t_degw[:])
            nc.scalar.activation(t_dinw[:], t_dinw[:],
                                 mybir.ActivationFunctionType.Sqrt)

            # ---- replicate packed gather indices to 128 partitions
            for blk in range(NBLK):
                for kk in range(8):
                    nc.sync.dma_start(
                        idxr[blk].ap()[:, 16 * kk : 16 * kk + 16, :],
                        idxt[blk].ap()[:, :, :],
                    )

            # ---- layer-1 table shard: shard1 = (dinv * x) @ W1
            for bt in range(WPC):
                t_x16 = bpool.tile([128, F], dt.float16, tag="x16")
                nc.sync.dma_start(t_x16[:], xs16.ap()[bt * 128 : (bt + 1) * 128, :])
                t_x = bpool.tile([128, F], dt.float32, tag="x")
                nc.vector.tensor_copy(t_x[:], t_x16[:])
                t_xs = bpool.tile([128, F], dt.float32, tag="xs")
                nc.vector.tensor_scalar(
                    t_xs[:], t_x[:], t_dinw[:, bt : bt + 1], None,
                    mybir.AluOpType.mult,
                )
                p_xT = pps.tile([128, 128], dt.float32, tag="xT")
                nc.tensor.transpose(p_xT[:], t_xs[:], t_id[:])
                t_xsT = bpool.tile([128, F], dt.float32, tag="xsT")
                nc.vector.tensor_copy(t_xsT[:], p_xT[:])
                p_h = pps.tile([128, F], dt.float32, tag="h")
                nc.tensor.matmul(p_h[:], t_xsT[:], t_w1[:], start=True, stop=True)
                t_h = bpool.tile([128, F], dt.float32, tag="h")
                nc.vector.tensor_copy(t_h[:], p_h[:])
                nc.sync.dma_start(shard1.ap()[bt * 128 : (bt + 1) * 128, :], t_h[:])

            # ---- publish full layer-1 table
            tc.strict_bb_all_engine_barrier()
            nc.gpsimd.collective_compute(
                "AllGather", mybir.AluOpType.bypass, replica_groups=groups,
                ins=[shard1.ap().opt()], outs=[table.ap().opt()],
            )
            tc.strict_bb_all_engine_barrier()

            def gather_layer(last):
                """Gather+aggregate own windows from `table`.

                last=False: epilogue fuses the layer-2 shard build into
                shard2.  last=True: epilogue writes the fp16 output.
                """
                t_bias = t_b2 if last else t_b1
                for b in range(NB):
                    t_msg = mpool.tile([128, B * tw, F], dt.float32, tag="msg")
                    t_lid16 = spool.tile([128, B * tw], dt.float16, tag="lid16")
                    nc.sync.dma_start(t_lid16[:], lidt.ap()[b, :, :])
                    t_lid = spool.tile([128, B * tw], dt.float32, tag="lid")
                    nc.vector.tensor_copy(t_lid[:], t_lid16[:])
                    for blk in range(NBLK):
                        cap = caps[blk]
                        t_ix = spool.tile([128, (B * cap) // 16], dt.int16,
                                          tag=f"ix{blk}")
                        nc.sync.dma_start(t_ix[:], idxr[blk].ap()[b, :, :])
                        t0 = B * btb[blk]
                        nc.gpsimd.dma_gather(
                            t_msg[:, t0 : t0 + (B * cap) // 128, :],
                            table.ap()[bases[blk] : bases[blk] + sizes[blk], :],
                            t_ix[:],
                            B * cap, B * cap, F,
                            single_packet=False,
                        )
                    for r in range(B):
                        k = b * B + r          # window index within core
                        p_agg = pps.tile([128, F], dt.float32, tag="agg")
                        wt = _win_tiles(cfg, r)
                        for jj, t in enumerate(wt):
                            t_S = spool.tile([128, 128], dt.float32, tag="S")
                            nc.vector.tensor_scalar(
                                t_S[:], t_iota[:], t_lid[:, t : t + 1], None,
                                mybir.AluOpType.is_equal,
                            )
                            nc.tensor.matmul(
                                p_agg[:], t_S[:], t_msg[:, t, :],
                                start=(jj == 0), stop=(jj == len(wt) - 1),
                            )
                        t_e = epool.tile([128, F], dt.float32, tag="e")
                        nc.vector.tensor_scalar(
                            t_e[:], p_agg[:], t_dinw[:, k : k + 1], None,
                            mybir.AluOpType.mult,
                        )
                        nc.vector.tensor_tensor(
                            t_e[:], t_e[:], t_bias[:], mybir.AluOpType.add
                        )
                        t_h = epool.tile([128, F], dt.float32, tag="h")
                        nc.scalar.activation(
                            t_h[:], t_e[:], mybir.ActivationFunctionType.Relu
                        )
                        if last:
                            t_o16 = epool.tile([128, F], dt.float16, tag="o16")
                            nc.vector.tensor_copy(t_o16[:], t_h[:])
                            nc.sync.dma_start(
                                out16.ap()[k * 128 : (k + 1) * 128, :], t_o16[:]
                            )
                        else:
                            # fused layer-2 shard build: (dinv*h) @ W2
                            t_hs = epool.tile([128, F], dt.float32, tag="hs")
                            nc.vector.tensor_scalar(
                                t_hs[:], t_h[:], t_dinw[:, k : k + 1], None,
                                mybir.AluOpType.mult,
                            )
                            p_hT = pps.tile([128, 128], dt.float32, tag="xT")
                            nc.tensor.transpose(p_hT[:], t_hs[:], t_id[:])
                            t_hT = epool.tile([128, F], dt.float32, tag="hT")
                            nc.vector.tensor_copy(t_hT[:], p_hT[:])
                            p_h2 = pps.tile([128, F], dt.float32, tag="h")
                            nc.tensor.matmul(p_h2[:], t_hT[:], t_w2[:],
                                             start=True, stop=True)
                            t_h2 = epool.tile([128, F], dt.float32, tag="h2")
                            nc.vector.tensor_copy(t_h2[:], p_h2[:])
                            nc.sync.dma_start(
                                shard2.ap()[k * 128 : (k + 1) * 128, :], t_h2[:]
                            )

            gather_layer(last=False)

            # ---- publish full layer-2 table (reuses `table`)
            tc.strict_bb_all_engine_barrier()
            nc.gpsimd.collective_compute(
                "AllGather", mybir.AluOpType.bypass, replica_groups=groups,
                ins=[shard2.ap().opt()], outs=[table.ap().opt()],
            )
            tc.strict_bb_all_engine_barrier()

            gather_layer(last=True)

    nc.compile()
    return nc


def kernel(x, edge_index, W1, b1, W2, b2):
    global _compiled, _last_wall_s
    x = np.asarray(x, np.float32)
    edge_index = np.asarray(edge_index)
    cfg, data = _host_prep(edge_index)
    if _compiled is None or _compiled[1] != cfg:
        _compiled = (_build_nc(cfg), cfg)
    nc = _compiled[0]

    from concourse.bass_utils import run_bass_kernel_spmd

    xpad = np.zeros((NPAD, F), np.float16)
    xpad[:N] = x.astype(np.float16)
    iota = np.tile(np.arange(128, dtype=np.float32)[None, :], (128, 1))
    ident = np.eye(128, dtype=np.float32)
    bt1 = np.tile(np.asarray(b1, np.float32)[None, :], (128, 1))
    bt2 = np.tile(np.asarray(b2, np.float32)[None, :], (128, 1))
    w1 = np.asarray(W1, np.float32)
    w2 = np.asarray(W2, np.float32)
    in_maps = []
    for c in range(NCORES):
        m = {
            "xs16": xpad[c * SH : (c + 1) * SH],
            "wmat1": w1, "wmat2": w2, "btile1": bt1, "btile2": bt2,
            "iota": iota, "ident": ident,
            "degw": data["degT"][:, c * WPC : (c + 1) * WPC].copy(),
            "lids": data["lids"][c],
        }
        for blk in range(NBLK):
            m[f"idx{blk}"] = data["idxs"][blk][c]
        in_maps.append(m)

    t0 = time.time()
    res = run_bass_kernel_spmd(nc, in_maps, core_ids=list(range(NCORES)),
                               trace=False)
    _last_wall_s = time.time() - t0
    out = np.concatenate([res.results[c]["out16"] for c in range(NCORES)],
                         axis=0)
    return out[:N].astype(np.float32)


# revision 5
# speedup vs baseline: 22.1172x; 3.0494x over previous
"""2-layer GCN encoder on 8 Trainium2 NeuronCores (Bass/Tile), single-shot.

Math: out = relu(Dinv (A+I) Dinv (x W) + b) twice, Dinv = deg^-1/2.
Factored as: table = (dinv * x) @ W ; agg[v] = sum_{e: dst=v} table[src_e] ;
out[v] = relu(dinv[v] * agg[v] + b)   -- no per-edge weights needed.

Distribution: dst-node sharding, one device invocation for BOTH layers.
Node ids padded to 100352 = 784 windows of 128; core p owns 98 windows
(12544 rows). Each core receives only its own x rows (fp16), builds its
table shard (dinv*x)@W1, and the full table is assembled on-device with
an 8-core AllGather over NeuronLink. Layer-1 aggregation fuses the
layer-2 shard build ((dinv*relu(...))@W2) in its epilogue, a second
AllGather publishes it, and layer-2 aggregation writes the fp16 output
shard. Host traffic is ~65MB total vs ~930MB for the two-invocation
replicated-table variant; the inter-layer halo exchange never leaves the
device.

Gather indices are int16 (reach 32768), so sources are split into 4
blocks with per-block base offsets on the gather's table AP. Per
(window, block) the edge count is data-dependent while gather calls need
static shapes, so the host computes per-block caps (128-aligned) from
the actual graph and pads with repeats of block-row 0. Padded slots
carry lid = -1 so their one-hot column in S is all-zero and they
contribute nothing. Indices are shipped packed in 16 partitions and
replicated to the gather's 128-partition layout on-device (8 DRAM->DRAM
copies), cutting their host transfer 8x.

Slot layout per batch of B windows (block-major so each gather call's
slots are contiguous): [blk0: w0 cap0, w1 cap0 | blk1: w0 cap1, ...].
Segment-sum on the tensor engine: per 128-slot tile, S[e, j] =
(lid[e] == j) built by the vector engine, then psum[dst, feat] +=
S.T @ msgs accumulated over the window's tiles.
"""
import sys
sys.path.insert(0, "/opt/trn_rl_repo")

import math
import time
import numpy as np

N = 100000
F = 128
NCORES = 8
WIN = 128                      # dst nodes per window
NPAD = 100352                  # 784 * 128
NW = NPAD // WIN               # 784 windows
WPC = NW // NCORES             # 98 windows per core
SH = WPC * WIN                 # 12544 rows per core
BLOCK = 32768                  # gather idx block (int16 reach)
NBLK = 4                       # 3*32768 + 2048 = 100352
B = 2                          # windows per gather batch
NB = WPC // B                  # 49 batches

_compiled = None               # (nc, cfg) cache across invocations
_last_exec_ns = None           # filled when a real trace is available
_last_wall_s = None            # wall time of device calls (incl transfers)


def _host_prep(edge_index):
    """Build per-core gather indices / lids / caps (fully vectorized)."""
    e0 = np.asarray(edge_index[0], np.int64)
    e1 = np.asarray(edge_index[1], np.int64)
    loop = np.arange(N, dtype=np.int64)
    src = np.concatenate([e0, loop])
    dst = np.concatenate([e1, loop])
    deg = np.bincount(dst, minlength=NPAD).astype(np.float32)
    deg[N:] = 1.0

    g = src >> 15                                 # src block 0..3
    w = dst >> 7                                  # global window 0..783
    order = np.lexsort((src, g, w))               # by (window, block, src)
    src, dst, g, w = src[order], dst[order], g[order], w[order]

    grp = w * NBLK + g
    counts = np.bincount(grp, minlength=NW * NBLK).reshape(NW, NBLK)
    caps = [int(128 * math.ceil(max(int(counts[:, blk].max()), 1) / 128))
            for blk in range(NBLK)]
    tw = sum(caps) // 128                         # tiles per window
    btb = [0]
    for cap in caps:
        btb.append(btb[-1] + cap // 128)
    cum = np.concatenate([[0], np.cumsum(counts.reshape(-1))])

    j = np.arange(len(src)) - cum[grp]            # rank within (w, blk) run
    c = w // WPC                                  # owning core
    k = w % WPC                                   # window within core
    b = k // B                                    # gather batch
    r = k % B                                     # window within batch

    idxs = []
    lids_flat = np.full(NCORES * NB * 128 * (B * tw), -1.0, np.float16)
    lidv = (dst & 127).astype(np.float16)
    for blk in range(NBLK):
        m = g == blk
        cap = caps[blk]
        flat = np.zeros(NCORES * NB * B * cap, np.int64)
        addr = ((c[m] * NB + b[m]) * B + r[m]) * cap + j[m]
        flat[addr] = src[m] - blk * BLOCK         # in-block idx (< 32768)
        # [n] slot stream -> [16, n/16]: slot i -> (i%16, i//16)
        idxs.append(flat.reshape(NCORES, NB, (B * cap) // 16, 16)
                    .transpose(0, 1, 3, 2).astype(np.int16).copy())
        pos = r[m] * cap + j[m]                   # slot within batch stream
        laddr = (((c[m] * NB + b[m]) * 128 + (pos & 127)) * (B * tw)
                 + B * btb[blk] + (pos >> 7))
        lids_flat[laddr] = lidv[m]
    lids = lids_flat.reshape(NCORES, NB, 128, B * tw)

    cfg = {"caps": tuple(caps), "tw": int(tw), "btb": tuple(btb)}
    data = {"idxs": idxs, "lids": lids,
            "degT": deg.reshape(NW, 128).T.copy()}
    return cfg, data


def _win_tiles(cfg, r):
    """Tile indices (within a batch's tile grid) owned by window r."""
    caps, btb = cfg["caps"], cfg["btb"]
    tiles = []
    for blk in range(NBLK):
        cb = caps[blk] // 128
        base = B * btb[blk] + r * cb
        tiles.extend(range(base, base + cb))
    return tiles


def _build_nc(cfg):
    from concourse import bacc, mybir
    import concourse.tile as tile
    from concourse import library_config
    import contextlib

    dt = mybir.dt
    caps, tw, btb = cfg["caps"], cfg["tw"], cfg["btb"]
    bases = [0, BLOCK, 2 * BLOCK, 3 * BLOCK]
    sizes = [BLOCK, BLOCK, BLOCK, NPAD - 3 * BLOCK]

    nc = bacc.Bacc("TRN2", target_bir_lowering=False, debug=False,
                   num_devices=NCORES)
    xs16 = nc.dram_tensor("xs16", [SH, F], dt.float16, kind="ExternalInput")
    wmat1 = nc.dram_tensor("wmat1", [F, F], dt.float32, kind="ExternalInput")
    wmat2 = nc.dram_tensor("wmat2", [F, F], dt.float32, kind="ExternalInput")
    btile1 = nc.dram_tensor("btile1", [128, F], dt.float32, kind="ExternalInput")
    btile2 = nc.dram_tensor("btile2", [128, F], dt.float32, kind="ExternalInput")
    iota = nc.dram_tensor("iota", [128, 128], dt.float32, kind="ExternalInput")
    ident = nc.dram_tensor("ident", [128, 128], dt.float32, kind="ExternalInput")
    degw = nc.dram_tensor("degw", [128, WPC], dt.float32, kind="ExternalInput")
    idxt = [
        nc.dram_tensor(f"idx{blk}", [NB, 16, (B * caps[blk]) // 16], dt.int16,
                       kind="ExternalInput")
        for blk in range(NBLK)
    ]
    lidt = nc.dram_tensor("lids", [NB, 128, B * tw], dt.float16,
                          kind="ExternalInput")
    idxr = [
        nc.dram_tensor(f"idxr{blk}", [NB, 128, (B * caps[blk]) // 16],
                       dt.int16, kind="Internal")
        for blk in range(NBLK)
    ]
    shard1 = nc.dram_tensor("shard1", [SH, F], dt.float32, kind="Internal")
    shard2 = nc.dram_tensor("shard2", [SH, F], dt.float32, kind="Internal")
    table = nc.dram_tensor("table", [NPAD, F], dt.float32, kind="Internal",
                           addr_space="Shared")
    out16 = nc.dram_tensor("out16", [SH, F], dt.float16,
                           kind="ExternalOutput")

    groups = [list(range(NCORES))]

    with tile.TileContext(nc) as tc:
        ctx = contextlib.ExitStack()
        with ctx:
            cpool = ctx.enter_context(tc.tile_pool(name="const", bufs=1))
            bpool = ctx.enter_context(tc.tile_pool(name="build", bufs=3))
            mpool = ctx.enter_context(tc.tile_pool(name="msg", bufs=2))
            spool = ctx.enter_context(tc.tile_pool(name="sprep", bufs=6))
            epool = ctx.enter_context(tc.tile_pool(name="epi", bufs=3))
            pps = ctx.enter_context(tc.tile_pool(name="ps", bufs=2, space="PSUM"))

            nc.gpsimd.load_library(library_config.mlp)

            # ---- constants
            t_iota = cpool.tile([128, 128], dt.float32, tag="iota")
            nc.sync.dma_start(t_iota[:], iota.ap()[:, :])
            t_id = cpool.tile([128, 128], dt.float32, tag="ident")
            nc.sync.dma_start(t_id[:], ident.ap()[:, :])
            t_w1 = cpool.tile([F, F], dt.float32, tag="w1")
            nc.sync.dma_start(t_w1[:], wmat1.ap()[:, :])
            t_w2 = cpool.tile([F, F], dt.float32, tag="w2")
            nc.sync.dma_start(t_w2[:], wmat2.ap()[:, :])
            t_b1 = cpool.tile([128, F], dt.float32, tag="b1")
            nc.sync.dma_start(t_b1[:], btile1.ap()[:, :])
            t_b2 = cpool.tile([128, F], dt.float32, tag="b2")
            nc.sync.dma_start(t_b2[:], btile2.ap()[:, :])

            t_degw = cpool.tile([128, WPC], dt.float32, tag="degw")
            nc.sync.dma_start(t_degw[:], degw.ap()[:, :])
            t_dinw = cpool.tile([128, WPC], dt.float32, tag="dinw")
            nc.vector.reciprocal(t_dinw[:], t_degw[:])
            nc.scalar.activation(t_dinw[:], t_dinw[:],
                                 mybir.ActivationFunctionType.Sqrt)

            # ---- replicate packed gather indices to 128 partitions
            for blk in range(NBLK):
                for kk in range(8):
                    nc.sync.dma_start(
                        idxr[blk].ap()[:, 16 * kk : 16 * kk + 16, :],
                        idxt[blk].ap()[:, :, :],
                    )

            # ---- layer-1 table shard: shard1 = (dinv * x) @ W1
            for bt in range(WPC):
                t_x16 = bpool.tile([128, F], dt.float16, tag="x16")
                nc.sync.dma_start(t_x16[:], xs16.ap()[bt * 128 : (bt + 1) * 128, :])
                t_x = bpool.tile([128, F], dt.float32, tag="x")
                nc.vector.tensor_copy(t_x[:], t_x16[:])
                t_xs = bpool.tile([128, F], dt.float32, tag="xs")
                nc.vector.tensor_scalar(
                    t_xs[:], t_x[:], t_dinw[:, bt : bt + 1], None,
                    mybir.AluOpType.mult,
                )
                p_xT = pps.tile([128, 128], dt.float32, tag="xT")
                nc.tensor.transpose(p_xT[:], t_xs[:], t_id[:])
                t_xsT = bpool.tile([128, F], dt.float32, tag="xsT")
                nc.vector.tensor_copy(t_xsT[:], p_xT[:])
                p_h = pps.tile([128, F], dt.float32, tag="h")
                nc.tensor.matmul(p_h[:], t_xsT[:], t_w1[:], start=True, stop=True)
                t_h = bpool.tile([128, F], dt.float32, tag="h")
                nc.vector.tensor_copy(t_h[:], p_h[:])
                nc.sync.dma_start(shard1.ap()[bt * 128 : (bt + 1) * 128, :], t_h[:])

            # ---- publish full layer-1 table
            tc.strict_bb_all_engine_barrier()
            nc.gpsimd.collective_compute(
                "AllGather", mybir.AluOpType.bypass, replica_groups=groups,
                ins=[shard1.ap().opt()], outs=[table.ap().opt()],
            )
            tc.strict_bb_all_engine_barrier()

            def gather_layer(last):
                """Gather+aggregate own windows from `table`.

                last=False: epilogue fuses the layer-2 shard build into
                shard2.  last=True: epilogue writes the fp16 output.
                """
                t_bias = t_b2 if last else t_b1
                for b in range(NB):
                    t_msg = mpool.tile([128, B * tw, F], dt.float32, tag="msg")
                    t_lid16 = spool.tile([128, B * tw], dt.float16, tag="lid16")
                    nc.sync.dma_start(t_lid16[:], lidt.ap()[b, :, :])
                    t_lid = spool.tile([128, B * tw], dt.float32, tag="lid")
                    nc.vector.tensor_copy(t_lid[:], t_lid16[:])
                    for blk in range(NBLK):
                        cap = caps[blk]
                        t_ix = spool.tile([128, (B * cap) // 16], dt.int16,
                                          tag=f"ix{blk}")
                        nc.sync.dma_start(t_ix[:], idxr[blk].ap()[b, :, :])
                        t0 = B * btb[blk]
                        nc.gpsimd.dma_gather(
                            t_msg[:, t0 : t0 + (B * cap) // 128, :],
                            table.ap()[bases[blk] : bases[blk] + sizes[blk], :],
                            t_ix[:],
                            B * cap, B * cap, F,
                            single_packet=False,
                        )
                    for r in range(B):
                        k = b * B + r          # window index within core
                        p_agg = pps.tile([128, F], dt.float32, tag="agg")
                        wt = _win_tiles(cfg, r)
                        for jj, t in enumerate(wt):
                            t_S = spool.tile([128, 128], dt.float32, tag="S")
                            nc.vector.tensor_scalar(
                                t_S[:], t_iota[:], t_lid[:, t : t + 1], None,
                                mybir.AluOpType.is_equal,
                            )
                            nc.tensor.matmul(
                                p_agg[:], t_S[:], t_msg[:, t, :],
                                start=(jj == 0), stop=(jj == len(wt) - 1),
                            )
                        t_e = epool.tile([128, F], dt.float32, tag="e")
                        nc.vector.tensor_scalar(
                            t_e[:], p_agg[:], t_dinw[:, k : k + 1], None,
                            mybir.AluOpType.mult,
                        )
                        nc.vector.tensor_tensor(
                            t_e[:], t_e[:], t_bias[:], mybir.AluOpType.add
                        )
                        t_h = epool.tile([128, F], dt.float32, tag="h")
                        nc.scalar.activation(
                            t_h[:], t_e[:], mybir.ActivationFunctionType.Relu
                        )
                        if last:
                            t_o16 = epool.tile([128, F], dt.float16, tag="o16")
                            nc.vector.tensor_copy(t_o16[:], t_h[:])
                            nc.sync.dma_start(
                                out16.ap()[k * 128 : (k + 1) * 128, :], t_o16[:]
                            )
                        else:
                            # fused layer-2 shard build: (dinv*h) @ W2
                            t_hs = epool.tile([128, F], dt.float32, tag="hs")
                            nc.vector.tensor_scalar(
                                t_hs[:], t_h[:], t_dinw[:, k : k + 1], None,
                                mybir.AluOpType.mult,
                            )
                            p_hT = pps.tile([128, 128], dt.float32, tag="xT")
                            nc.tensor.transpose(p_hT[:], t_hs[:], t_id[:])
                            t_hT = epool.tile([128, F], dt.float32, tag="hT")
                            nc.vector.tensor_copy(t_hT[:], p_hT[:])
                            p_h2 = pps.tile([128, F], dt.float32, tag="h")
                            nc.tensor.matmul(p_h2[:], t_hT[:], t_w2[:],
                                             start=True, stop=True)
                            t_h2 = epool.tile([128, F], dt.float32, tag="h2")
                            nc.vector.tensor_copy(t_h2[:], p_h2[:])
                            nc.sync.dma_start(
                                shard2.ap()[k * 128 : (k + 1) * 128, :], t_h2[:]
                            )

            gather_layer(last=False)

            # ---- publish full layer-2 table (reuses `table`)
            tc.strict_bb_all_engine_barrier()
            nc.gpsimd.collective_compute(
                "AllGather", mybir.AluOpType.bypass, replica_groups=groups,
                ins=[shard2.ap().opt()], outs=[table.ap().opt()],
            )
            tc.strict_bb_all_engine_barrier()

            gather_layer(last=True)

    nc.compile()
    return nc


def _aot_compile(nc):
    """AOT-compile the 8-core SPMD executable (no data, no device calls
    beyond compilation). Returns everything needed to run it."""
    from concourse import bass2jax, mybir
    import jax
    import jax.numpy as jnp
    from jax.sharding import Mesh, PartitionSpec, NamedSharding
    from jax.experimental.shard_map import shard_map

    bass2jax.install_neuronx_cc_hook()
    partition_name = (nc.partition_id_tensor.name
                      if nc.partition_id_tensor else None)
    in_names, out_names, out_avals = [], [], []
    for alloc in nc.m.functions[0].allocations:
        if not isinstance(alloc, mybir.MemoryLocationSet):
            continue
        name = alloc.memorylocations[0].name
        if alloc.kind == "ExternalInput":
            if name != partition_name:
                in_names.append(name)
        elif alloc.kind == "ExternalOutput":
            out_names.append(name)
            out_avals.append(jax.core.ShapedArray(
                tuple(alloc.tensor_shape), mybir.dt.np(alloc.dtype)))
    n_params = len(in_names)
    n_outs = len(out_avals)
    in_names_all = (in_names + out_names
                    + ([partition_name] if partition_name else []))

    def _body(*args):
        operands = list(args)
        if partition_name is not None:
            operands.append(bass2jax.partition_id_tensor())
        outs = bass2jax._bass_exec_p.bind(
            *operands, out_avals=tuple(out_avals),
            in_names=tuple(in_names_all), out_names=tuple(out_names),
            lowering_input_output_aliases=(), sim_require_finite=True,
            sim_require_nnan=True, nc=nc)
        return tuple(outs)

    devices = jax.devices()[:NCORES]
    mesh = Mesh(np.asarray(devices), ("core",))
    spec = NamedSharding(mesh, PartitionSpec("core"))
    in_specs = (PartitionSpec("core"),) * (n_params + n_outs)
    out_specs = (PartitionSpec("core"),) * n_outs
    donate = tuple(range(n_params, n_params + n_outs))
    sharded = jax.jit(shard_map(_body, mesh=mesh, in_specs=in_specs,
                                out_specs=out_specs, check_rep=False),
                      donate_argnums=donate, keep_unused=True)

    # NOTE: per-core BIR shapes concat along axis 0 across the 8 cores
    def _glob(aval):
        return jax.ShapeDtypeStruct(
            (NCORES * aval.shape[0], *aval.shape[1:]), aval.dtype)

    in_structs = []   # filled by caller lookup via in_names order
    self_shapes = {}
    for alloc in nc.m.functions[0].allocations:
        if not isinstance(alloc, mybir.MemoryLocationSet):
            continue
        name = alloc.memorylocations[0].name
        if alloc.kind == "ExternalInput" and name != partition_name:
            self_shapes[name] = (tuple(alloc.tensor_shape),
                                 mybir.dt.np(alloc.dtype))
    for name in in_names:
        shape, dtype = self_shapes[name]
        in_structs.append(jax.ShapeDtypeStruct(
            (NCORES * shape[0], *shape[1:]), dtype))
    out_structs = [_glob(a) for a in out_avals]

    compiled = sharded.lower(*in_structs, *out_structs).compile()

    zero_fns = []
    for s in out_structs:
        zero_fns.append(
            jax.jit(lambda s=s: jnp.zeros(s.shape, s.dtype),
                    out_shardings=spec).lower().compile())

    return {"compiled": compiled, "zero_fns": zero_fns,
            "in_names": in_names, "out_names": out_names,
            "out_avals": out_avals, "spec": spec}


def kernel(x, edge_index, W1, b1, W2, b2):
    global _compiled, _last_wall_s
    import jax

    x = np.asarray(x, np.float32)
    edge_index = np.asarray(edge_index)
    cfg, data = _host_prep(edge_index)
    if _compiled is None or _compiled[1] != cfg:
        nc = _build_nc(cfg)
        _compiled = ((nc, _aot_compile(nc)), cfg)
    nc, rt = _compiled[0]

    xpad = np.zeros((NPAD, F), np.float16)
    xpad[:N] = x.astype(np.float16)
    iota = np.tile(np.arange(128, dtype=np.float32)[None, :], (128, 1))
    ident = np.eye(128, dtype=np.float32)
    bt1 = np.tile(np.asarray(b1, np.float32)[None, :], (128, 1))
    bt2 = np.tile(np.asarray(b2, np.float32)[None, :], (128, 1))
    w1 = np.asarray(W1, np.float32)
    w2 = np.asarray(W2, np.float32)
    in_maps = []
    for c in range(NCORES):
        m = {
            "xs16": xpad[c * SH : (c + 1) * SH],
            "wmat1": w1, "wmat2": w2, "btile1": bt1, "btile2": bt2,
            "iota": iota, "ident": ident,
            "degw": data["degT"][:, c * WPC : (c + 1) * WPC].copy(),
            "lids": data["lids"][c],
        }
        for blk in range(NBLK):
            m[f"idx{blk}"] = data["idxs"][blk][c]
        in_maps.append(m)

    # ---- timed device window: h2d + execute + d2h
    t0 = time.time()
    concat_in = [
        np.concatenate([np.asarray(in_maps[c][name]) for c in range(NCORES)],
                       axis=0)
        for name in rt["in_names"]
    ]
    shardings = rt["compiled"].input_shardings[0]
    dev_in = [jax.device_put(a, s) for a, s in zip(concat_in, shardings)]
    dev_zero = [zf() for zf in rt["zero_fns"]]
    out_arrs = rt["compiled"](*dev_in, *dev_zero)
    host_out = [np.asarray(o) for o in out_arrs]
    _last_wall_s = time.time() - t0

    out = host_out[rt["out_names"].index("out16")]
    return out[:N].astype(np.float32)


# revision 10
# speedup vs baseline: 22.2883x; 1.0077x over previous
"""2-layer GCN encoder on 8 Trainium2 NeuronCores (Bass/Tile), single-shot.

Math: out = relu(Dinv (A+I) Dinv (x W) + b) twice, Dinv = deg^-1/2.
Factored as: table = (dinv * x) @ W ; agg[v] = sum_{e: dst=v} table[src_e] ;
out[v] = relu(dinv[v] * agg[v] + b)   -- no per-edge weights needed.

Distribution: dst-node sharding, one device invocation for BOTH layers.
Node ids padded to 100352 = 784 windows of 128; core p owns 98 windows
(12544 rows). Each core receives only its own x rows (fp16), builds its
table shard (dinv*x)@W1, and the full table is assembled on-device with
an 8-core AllGather over NeuronLink. Layer-1 aggregation fuses the
layer-2 shard build ((dinv*relu(...))@W2) in its epilogue, a second
AllGather publishes it, and layer-2 aggregation writes the fp16 output
shard. Host traffic is ~65MB total vs ~930MB for the two-invocation
replicated-table variant; the inter-layer halo exchange never leaves the
device.

Gather indices are int16 (reach 32768), so sources are split into 4
blocks with per-block base offsets on the gather's table AP. Per
(window, block) the edge count is data-dependent while gather calls need
static shapes, so the host computes per-block caps (128-aligned) from
the actual graph and pads with repeats of block-row 0. Padded slots
carry lid = -1 so their one-hot column in S is all-zero and they
contribute nothing. Indices are shipped packed in 16 partitions and
replicated to the gather's 128-partition layout on-device (8 DRAM->DRAM
copies), cutting their host transfer 8x.

Slot layout per batch of B windows (block-major so each gather call's
slots are contiguous): [blk0: w0 cap0, w1 cap0 | blk1: w0 cap1, ...].
Segment-sum on the tensor engine: per 128-slot tile, S[e, j] =
(lid[e] == j) built by the vector engine, then psum[dst, feat] +=
S.T @ msgs accumulated over the window's tiles.
"""
import sys
sys.path.insert(0, "/opt/trn_rl_repo")

import math
import time
import numpy as np

N = 100000
F = 128
NCORES = 8
WIN = 128                      # dst nodes per window
NPAD = 100352                  # 784 * 128
NW = NPAD // WIN               # 784 windows
WPC = NW // NCORES             # 98 windows per core
SH = WPC * WIN                 # 12544 rows per core
BLOCK = 32768                  # gather idx block (int16 reach)
NBLK = 4                       # 3*32768 + 2048 = 100352
B = 2                          # windows per gather batch
NB = WPC // B                  # 49 batches

_compiled = None               # (nc, cfg) cache across invocations
_last_exec_ns = None           # filled when a real trace is available
_last_wall_s = None            # wall time of device calls (incl transfers)


def _host_prep(edge_index):
    """Build per-core gather indices / lids / caps (fully vectorized)."""
    e0 = np.asarray(edge_index[0], np.int64)
    e1 = np.asarray(edge_index[1], np.int64)
    loop = np.arange(N, dtype=np.int64)
    src = np.concatenate([e0, loop])
    dst = np.concatenate([e1, loop])
    deg = np.bincount(dst, minlength=NPAD).astype(np.float32)
    deg[N:] = 1.0

    g = src >> 15                                 # src block 0..3
    w = dst >> 7                                  # global window 0..783
    order = np.lexsort((src, g, w))               # by (window, block, src)
    src, dst, g, w = src[order], dst[order], g[order], w[order]

    grp = w * NBLK + g
    counts = np.bincount(grp, minlength=NW * NBLK).reshape(NW, NBLK)
    caps = [int(128 * math.ceil(max(int(counts[:, blk].max()), 1) / 128))
            for blk in range(NBLK)]
    tw = sum(caps) // 128                         # tiles per window
    btb = [0]
    for cap in caps:
        btb.append(btb[-1] + cap // 128)
    cum = np.concatenate([[0], np.cumsum(counts.reshape(-1))])

    j = np.arange(len(src)) - cum[grp]            # rank within (w, blk) run
    c = w // WPC                                  # owning core
    k = w % WPC                                   # window within core
    b = k // B                                    # gather batch
    r = k % B                                     # window within batch

    idxs = []
    lids_flat = np.full(NCORES * NB * 128 * (B * tw), -1.0, np.float16)
    lidv = (dst & 127).astype(np.float16)
    for blk in range(NBLK):
        m = g == blk
        cap = caps[blk]
        flat = np.zeros(NCORES * NB * B * cap, np.int64)
        addr = ((c[m] * NB + b[m]) * B + r[m]) * cap + j[m]
        flat[addr] = src[m] - blk * BLOCK         # in-block idx (< 32768)
        # [n] slot stream -> [16, n/16]: slot i -> (i%16, i//16)
        idxs.append(flat.reshape(NCORES, NB, (B * cap) // 16, 16)
                    .transpose(0, 1, 3, 2).astype(np.int16).copy())
        pos = r[m] * cap + j[m]                   # slot within batch stream
        laddr = (((c[m] * NB + b[m]) * 128 + (pos & 127)) * (B * tw)
                 + B * btb[blk] + (pos >> 7))
        lids_flat[laddr] = lidv[m]
    lids = lids_flat.reshape(NCORES, NB, 128, B * tw)

    cfg = {"caps": tuple(caps), "tw": int(tw), "btb": tuple(btb)}
    data = {"idxs": idxs, "lids": lids,
            "degT": deg.reshape(NW, 128).T.copy()}
    return cfg, data


def _win_tiles(cfg, r):
    """Tile indices (within a batch's tile grid) owned by window r."""
    caps, btb = cfg["caps"], cfg["btb"]
    tiles = []
    for blk in range(NBLK):
        cb = caps[blk] // 128
        base = B * btb[blk] + r * cb
        tiles.extend(range(base, base + cb))
    return tiles


def _build_nc(cfg):
    from concourse import bacc, bass, mybir
    import concourse.tile as tile
    from concourse import library_config
    import contextlib

    dt = mybir.dt
    caps, tw, btb = cfg["caps"], cfg["tw"], cfg["btb"]
    bases = [0, BLOCK, 2 * BLOCK, 3 * BLOCK]
    sizes = [BLOCK, BLOCK, BLOCK, NPAD - 3 * BLOCK]

    nc = bacc.Bacc("TRN2", target_bir_lowering=False, debug=False,
                   num_devices=NCORES)
    xs16 = nc.dram_tensor("xs16", [SH, F], dt.float16, kind="ExternalInput")
    wmat1 = nc.dram_tensor("wmat1", [F, F], dt.float32, kind="ExternalInput")
    wmat2 = nc.dram_tensor("wmat2", [F, F], dt.float32, kind="ExternalInput")
    btile1 = nc.dram_tensor("btile1", [128, F], dt.float32, kind="ExternalInput")
    btile2 = nc.dram_tensor("btile2", [128, F], dt.float32, kind="ExternalInput")
    iota = nc.dram_tensor("iota", [128, 128], dt.float32, kind="ExternalInput")
    ident = nc.dram_tensor("ident", [128, 128], dt.float32, kind="ExternalInput")
    degw = nc.dram_tensor("degw", [128, WPC], dt.float32, kind="ExternalInput")
    idxt = [
        nc.dram_tensor(f"idx{blk}", [NB, 16, (B * caps[blk]) // 16], dt.int16,
                       kind="ExternalInput")
        for blk in range(NBLK)
    ]
    lidt = nc.dram_tensor("lids", [NB, 128, B * tw], dt.float16,
                          kind="ExternalInput")
    idxr = [
        nc.dram_tensor(f"idxr{blk}", [NB, 128, (B * caps[blk]) // 16],
                       dt.int16, kind="Internal")
        for blk in range(NBLK)
    ]
    shard1 = nc.dram_tensor("shard1", [SH, F], dt.float32, kind="Internal")
    shard2 = nc.dram_tensor("shard2", [SH, F], dt.float32, kind="Internal")
    table = nc.dram_tensor("table", [NPAD, F], dt.float32, kind="Internal",
                           addr_space="Shared")
    out16 = nc.dram_tensor("out16", [SH, F], dt.float16,
                           kind="ExternalOutput")

    groups = [list(range(NCORES))]

    with tile.TileContext(nc) as tc:
        ctx = contextlib.ExitStack()
        with ctx:
            cpool = ctx.enter_context(tc.tile_pool(name="const", bufs=1))
            bpool = ctx.enter_context(tc.tile_pool(name="build", bufs=3))
            mpool = ctx.enter_context(tc.tile_pool(name="msg", bufs=2))
            spool = ctx.enter_context(tc.tile_pool(name="sprep", bufs=6))
            Spool = ctx.enter_context(tc.tile_pool(name="onehot", bufs=2))
            epool = ctx.enter_context(tc.tile_pool(name="epi", bufs=3))
            pps = ctx.enter_context(tc.tile_pool(name="ps", bufs=2, space="PSUM"))

            nc.gpsimd.load_library(library_config.mlp)

            # ---- constants
            t_iota = cpool.tile([128, 128], dt.float32, tag="iota")
            nc.sync.dma_start(t_iota[:], iota.ap()[:, :])
            t_id = cpool.tile([128, 128], dt.float32, tag="ident")
            nc.sync.dma_start(t_id[:], ident.ap()[:, :])
            t_w1 = cpool.tile([F, F], dt.float32, tag="w1")
            nc.sync.dma_start(t_w1[:], wmat1.ap()[:, :])
            t_w2 = cpool.tile([F, F], dt.float32, tag="w2")
            nc.sync.dma_start(t_w2[:], wmat2.ap()[:, :])
            t_b1 = cpool.tile([128, F], dt.float32, tag="b1")
            nc.sync.dma_start(t_b1[:], btile1.ap()[:, :])
            t_b2 = cpool.tile([128, F], dt.float32, tag="b2")
            nc.sync.dma_start(t_b2[:], btile2.ap()[:, :])

            t_degw = cpool.tile([128, WPC], dt.float32, tag="degw")
            nc.sync.dma_start(t_degw[:], degw.ap()[:, :])
            t_dinw = cpool.tile([128, WPC], dt.float32, tag="dinw")
            nc.vector.reciprocal(t_dinw[:], t_degw[:])
            nc.scalar.activation(t_dinw[:], t_dinw[:],
                                 mybir.ActivationFunctionType.Sqrt)

            # ---- replicate packed gather indices to 128 partitions
            for blk in range(NBLK):
                for kk in range(8):
                    nc.sync.dma_start(
                        idxr[blk].ap()[:, 16 * kk : 16 * kk + 16, :],
                        idxt[blk].ap()[:, :, :],
                    )

            # ---- layer-1 table shard: shard1 = (dinv * x) @ W1
            def build_win(bt):
                """bt: window index, ScalarValue expr or int."""
                t_x16 = bpool.tile([128, F], dt.float16, tag="x16")
                nc.sync.dma_start(t_x16[:], xs16.ap()[bass.ds(bt * 128, 128), :])
                t_x = bpool.tile([128, F], dt.float32, tag="x")
                nc.vector.tensor_copy(t_x[:], t_x16[:])
                t_xs = bpool.tile([128, F], dt.float32, tag="xs")
                nc.vector.tensor_scalar(
                    t_xs[:], t_x[:], t_dinw[:, bass.ds(bt, 1)], None,
                    mybir.AluOpType.mult,
                )
                p_xT = pps.tile([128, 128], dt.float32, tag="xT")
                nc.tensor.transpose(p_xT[:], t_xs[:], t_id[:])
                t_xsT = bpool.tile([128, F], dt.float32, tag="xsT")
                nc.vector.tensor_copy(t_xsT[:], p_xT[:])
                p_h = pps.tile([128, F], dt.float32, tag="h")
                nc.tensor.matmul(p_h[:], t_xsT[:], t_w1[:], start=True, stop=True)
                t_h = bpool.tile([128, F], dt.float32, tag="h")
                nc.vector.tensor_copy(t_h[:], p_h[:])
                nc.sync.dma_start(shard1.ap()[bass.ds(bt * 128, 128), :], t_h[:])

            with tc.For_i(0, WPC, 2) as bt:
                build_win(bt)
                build_win(bt + 1)

            # ---- publish full layer-1 table
            tc.strict_bb_all_engine_barrier()
            nc.gpsimd.collective_compute(
                "AllGather", mybir.AluOpType.bypass, replica_groups=groups,
                ins=[shard1.ap().opt()], outs=[table.ap().opt()],
            )
            tc.strict_bb_all_engine_barrier()

            def gather_batch(b, last):
                """Process gather batch b (ScalarValue expr or int).

                last=False: epilogue fuses the layer-2 shard build into
                shard2.  last=True: epilogue writes the fp16 output.
                """
                t_bias = t_b2 if last else t_b1
                t_msg = mpool.tile([128, B * tw, F], dt.float32, tag="msg")
                t_lid16 = spool.tile([128, B * tw], dt.float16, tag="lid16")
                nc.sync.dma_start(t_lid16[:], lidt.ap()[b, :, :])
                t_lid = spool.tile([128, B * tw], dt.float32, tag="lid")
                nc.vector.tensor_copy(t_lid[:], t_lid16[:])
                # all one-hot tiles for the batch in one broadcast is_equal:
                # S[p, t, j] = (lid[p, t] == j)
                t_S = Spool.tile([128, B * tw, 128], dt.float32, tag="S")
                nc.vector.tensor_tensor(
                    t_S[:],
                    t_lid[:].broadcast_to([128, B * tw, 128]),
                    t_iota[:, None, :].broadcast_to([128, B * tw, 128]),
                    mybir.AluOpType.is_equal,
                )
                for blk in range(NBLK):
                    cap = caps[blk]
                    t_ix = spool.tile([128, (B * cap) // 16], dt.int16,
                                      tag=f"ix{blk}")
                    nc.sync.dma_start(t_ix[:], idxr[blk].ap()[b, :, :])
                    t0 = B * btb[blk]
                    nc.gpsimd.dma_gather(
                        t_msg[:, t0 : t0 + (B * cap) // 128, :],
                        table.ap()[bases[blk] : bases[blk] + sizes[blk], :],
                        t_ix[:],
                        B * cap, B * cap, F,
                        single_packet=False,
                    )
                for r in range(B):
                    k = b * B + r              # window index within core
                    p_agg = pps.tile([128, F], dt.float32, tag="agg")
                    wt = _win_tiles(cfg, r)
                    for jj, t in enumerate(wt):
                        nc.tensor.matmul(
                            p_agg[:], t_S[:, t, :], t_msg[:, t, :],
                            start=(jj == 0), stop=(jj == len(wt) - 1),
                        )
                    t_e = epool.tile([128, F], dt.float32, tag="e")
                    nc.vector.tensor_scalar(
                        t_e[:], p_agg[:], t_dinw[:, bass.ds(k, 1)], None,
                        mybir.AluOpType.mult,
                    )
                    nc.vector.tensor_tensor(
                        t_e[:], t_e[:], t_bias[:], mybir.AluOpType.add
                    )
                    t_h = epool.tile([128, F], dt.float32, tag="h")
                    nc.scalar.activation(
                        t_h[:], t_e[:], mybir.ActivationFunctionType.Relu
                    )
                    if last:
                        t_o16 = epool.tile([128, F], dt.float16, tag="o16")
                        nc.vector.tensor_copy(t_o16[:], t_h[:])
                        nc.sync.dma_start(
                            out16.ap()[bass.ds(k * 128, 128), :], t_o16[:]
                        )
                    else:
                        # fused layer-2 shard build: (dinv*h) @ W2
                        t_hs = epool.tile([128, F], dt.float32, tag="hs")
                        nc.vector.tensor_scalar(
                            t_hs[:], t_h[:], t_dinw[:, bass.ds(k, 1)], None,
                            mybir.AluOpType.mult,
                        )
                        p_hT = pps.tile([128, 128], dt.float32, tag="xT")
                        nc.tensor.transpose(p_hT[:], t_hs[:], t_id[:])
                        t_hT = epool.tile([128, F], dt.float32, tag="hT")
                        nc.vector.tensor_copy(t_hT[:], p_hT[:])
                        p_h2 = pps.tile([128, F], dt.float32, tag="h")
                        nc.tensor.matmul(p_h2[:], t_hT[:], t_w2[:],
                                         start=True, stop=True)
                        t_h2 = epool.tile([128, F], dt.float32, tag="h2")
                        nc.vector.tensor_copy(t_h2[:], p_h2[:])
                        nc.sync.dma_start(
                            shard2.ap()[bass.ds(k * 128, 128), :], t_h2[:]
                        )

            def gather_layer(last):
                # NB = 49: unrolled-by-2 hardware loop over 48 + static tail
                with tc.For_i(0, NB - 1, 2) as b:
                    gather_batch(b, last)
                    gather_batch(b + 1, last)
                gather_batch(NB - 1, last)

            gather_layer(last=False)

            # ---- publish full layer-2 table (reuses `table`)
            tc.strict_bb_all_engine_barrier()
            nc.gpsimd.collective_compute(
                "AllGather", mybir.AluOpType.bypass, replica_groups=groups,
                ins=[shard2.ap().opt()], outs=[table.ap().opt()],
            )
            tc.strict_bb_all_engine_barrier()

            gather_layer(last=True)

    nc.compile()
    return nc


def _aot_compile(nc):
    """AOT-compile the 8-core SPMD executable (no data, no device calls
    beyond compilation). Returns everything needed to run it."""
    from concourse import bass2jax, mybir
    import jax
    import jax.numpy as jnp
    from jax.sharding import Mesh, PartitionSpec, NamedSharding
    from jax.experimental.shard_map import shard_map

    bass2jax.install_neuronx_cc_hook()
    partition_name = (nc.partition_id_tensor.name
                      if nc.partition_id_tensor else None)
    in_names, out_names, out_avals = [], [], []
    for alloc in nc.m.functions[0].allocations:
        if not isinstance(alloc, mybir.MemoryLocationSet):
            continue
        name = alloc.memorylocations[0].name
        if alloc.kind == "ExternalInput":
            if name != partition_name:
                in_names.append(name)
        elif alloc.kind == "ExternalOutput":
            out_names.append(name)
            out_avals.append(jax.core.ShapedArray(
                tuple(alloc.tensor_shape), mybir.dt.np(alloc.dtype)))
    n_params = len(in_names)
    n_outs = len(out_avals)
    in_names_all = (in_names + out_names
                    + ([partition_name] if partition_name else []))

    def _body(*args):
        operands = list(args)
        if partition_name is not None:
            operands.append(bass2jax.partition_id_tensor())
        outs = bass2jax._bass_exec_p.bind(
            *operands, out_avals=tuple(out_avals),
            in_names=tuple(in_names_all), out_names=tuple(out_names),
            lowering_input_output_aliases=(), sim_require_finite=True,
            sim_require_nnan=True, nc=nc)
        return tuple(outs)

    devices = jax.devices()[:NCORES]
    mesh = Mesh(np.asarray(devices), ("core",))
    spec = NamedSharding(mesh, PartitionSpec("core"))
    in_specs = (PartitionSpec("core"),) * (n_params + n_outs)
    out_specs = (PartitionSpec("core"),) * n_outs
    donate = tuple(range(n_params, n_params + n_outs))
    sharded = jax.jit(shard_map(_body, mesh=mesh, in_specs=in_specs,
                                out_specs=out_specs, check_rep=False),
                      donate_argnums=donate, keep_unused=True)

    # NOTE: per-core BIR shapes concat along axis 0 across the 8 cores
    def _glob(aval):
        return jax.ShapeDtypeStruct(
            (NCORES * aval.shape[0], *aval.shape[1:]), aval.dtype)

    in_structs = []   # filled by caller lookup via in_names order
    self_shapes = {}
    for alloc in nc.m.functions[0].allocations:
        if not isinstance(alloc, mybir.MemoryLocationSet):
            continue
        name = alloc.memorylocations[0].name
        if alloc.kind == "ExternalInput" and name != partition_name:
            self_shapes[name] = (tuple(alloc.tensor_shape),
                                 mybir.dt.np(alloc.dtype))
    for name in in_names:
        shape, dtype = self_shapes[name]
        in_structs.append(jax.ShapeDtypeStruct(
            (NCORES * shape[0], *shape[1:]), dtype))
    out_structs = [_glob(a) for a in out_avals]

    compiled = sharded.lower(*in_structs, *out_structs).compile()

    zero_fns = []
    for s in out_structs:
        zero_fns.append(
            jax.jit(lambda s=s: jnp.zeros(s.shape, s.dtype),
                    out_shardings=spec).lower().compile())

    return {"compiled": compiled, "zero_fns": zero_fns,
            "in_names": in_names, "out_names": out_names,
            "out_avals": out_avals, "spec": spec}


def kernel(x, edge_index, W1, b1, W2, b2):
    global _compiled, _last_wall_s
    import jax

    x = np.asarray(x, np.float32)
    edge_index = np.asarray(edge_index)
    cfg, data = _host_prep(edge_index)
    if _compiled is None or _compiled[1] != cfg:
        nc = _build_nc(cfg)
        _compiled = ((nc, _aot_compile(nc)), cfg)
    nc, rt = _compiled[0]

    xpad = np.zeros((NPAD, F), np.float16)
    xpad[:N] = x.astype(np.float16)
    iota = np.tile(np.arange(128, dtype=np.float32)[None, :], (128, 1))
    ident = np.eye(128, dtype=np.float32)
    bt1 = np.tile(np.asarray(b1, np.float32)[None, :], (128, 1))
    bt2 = np.tile(np.asarray(b2, np.float32)[None, :], (128, 1))
    w1 = np.asarray(W1, np.float32)
    w2 = np.asarray(W2, np.float32)
    in_maps = []
    for c in range(NCORES):
        m = {
            "xs16": xpad[c * SH : (c + 1) * SH],
            "wmat1": w1, "wmat2": w2, "btile1": bt1, "btile2": bt2,
            "iota": iota, "ident": ident,
            "degw": data["degT"][:, c * WPC : (c + 1) * WPC].copy(),
            "lids": data["lids"][c],
        }
        for blk in range(NBLK):
            m[f"idx{blk}"] = data["idxs"][blk][c]
        in_maps.append(m)

    # ---- timed device window: h2d + execute + d2h
    t0 = time.time()
    concat_in = [
        np.concatenate([np.asarray(in_maps[c][name]) for c in range(NCORES)],
                       axis=0)
        for name in rt["in_names"]
    ]
    shardings = rt["compiled"].input_shardings[0]
    dev_in = [jax.device_put(a, s) for a, s in zip(concat_in, shardings)]
    dev_zero = [zf() for zf in rt["zero_fns"]]
    out_arrs = rt["compiled"](*dev_in, *dev_zero)
    host_out = [np.asarray(o) for o in out_arrs]
    _last_wall_s = time.time() - t0

    out = host_out[rt["out_names"].index("out16")]
    return out[:N].astype(np.float32)


# revision 15
# speedup vs baseline: 22.3808x; 1.0041x over previous
"""2-layer GCN encoder on 8 Trainium2 NeuronCores (Bass/Tile), single-shot.

Math: out = relu(Dinv (A+I) Dinv (x W) + b) twice, Dinv = deg^-1/2.
Factored as: table = (dinv * x) @ W ; agg[v] = sum_{e: dst=v} table[src_e] ;
out[v] = relu(dinv[v] * agg[v] + b)   -- no per-edge weights needed.

Distribution: dst-node sharding, one device invocation for BOTH layers.
Node ids padded to 100352 = 784 windows of 128; core p owns 98 windows
(12544 rows). Each core receives only its own x rows (fp16), builds its
table shard (dinv*x)@W1, and the full table is assembled on-device with
an 8-core AllGather over NeuronLink. Layer-1 aggregation fuses the
layer-2 shard build ((dinv*relu(...))@W2) in its epilogue, a second
AllGather publishes it, and layer-2 aggregation writes the fp16 output
shard. Host traffic is ~65MB total vs ~930MB for the two-invocation
replicated-table variant; the inter-layer halo exchange never leaves the
device.

Gather indices are int16 (reach 32768), so sources are split into 4
blocks with per-block base offsets on the gather's table AP. Per
(window, block) the edge count is data-dependent while gather calls need
static shapes, so the host computes per-block caps (128-aligned) from
the actual graph and pads with repeats of block-row 0. Padded slots
carry lid = -1 so their one-hot column in S is all-zero and they
contribute nothing. Indices are shipped packed in 16 partitions and
replicated to the gather's 128-partition layout on-device (8 DRAM->DRAM
copies), cutting their host transfer 8x.

Slot layout per batch of B windows (block-major so each gather call's
slots are contiguous): [blk0: w0 cap0, w1 cap0 | blk1: w0 cap1, ...].
Segment-sum on the tensor engine: per 128-slot tile, S[e, j] =
(lid[e] == j) built by the vector engine, then psum[dst, feat] +=
S.T @ msgs accumulated over the window's tiles.
"""
import sys
sys.path.insert(0, "/opt/trn_rl_repo")

import math
import time
import numpy as np

N = 100000
F = 128
NCORES = 8
WIN = 128                      # dst nodes per window
NPAD = 100352                  # 784 * 128
NW = NPAD // WIN               # 784 windows
WPC = NW // NCORES             # 98 windows per core
SH = WPC * WIN                 # 12544 rows per core
BLOCK = 32768                  # gather idx block (int16 reach)
NBLK = 4                       # 3*32768 + 2048 = 100352
B = 2                          # windows per gather batch
NB = WPC // B                  # 49 batches

_compiled = None               # (nc, cfg) cache across invocations
_last_exec_ns = None           # filled when a real trace is available
_last_wall_s = None            # wall time of device calls (incl transfers)


def _host_prep(edge_index):
    """Build per-core gather indices / lids / caps (fully vectorized)."""
    e0 = np.asarray(edge_index[0], np.int64)
    e1 = np.asarray(edge_index[1], np.int64)
    loop = np.arange(N, dtype=np.int64)
    src = np.concatenate([e0, loop])
    dst = np.concatenate([e1, loop])
    deg = np.bincount(dst, minlength=NPAD).astype(np.float32)
    deg[N:] = 1.0

    g = src >> 15                                 # src block 0..3
    w = dst >> 7                                  # global window 0..783
    order = np.lexsort((src, g, w))               # by (window, block, src)
    src, dst, g, w = src[order], dst[order], g[order], w[order]

    grp = w * NBLK + g
    counts = np.bincount(grp, minlength=NW * NBLK).reshape(NW, NBLK)
    caps = [int(128 * math.ceil(max(int(counts[:, blk].max()), 1) / 128))
            for blk in range(NBLK)]
    tw = sum(caps) // 128                         # tiles per window
    btb = [0]
    for cap in caps:
        btb.append(btb[-1] + cap // 128)
    cum = np.concatenate([[0], np.cumsum(counts.reshape(-1))])

    j = np.arange(len(src)) - cum[grp]            # rank within (w, blk) run
    c = w // WPC                                  # owning core
    k = w % WPC                                   # window within core
    b = k // B                                    # gather batch
    r = k % B                                     # window within batch

    idxs = []
    lids_flat = np.full(NCORES * NB * 128 * (B * tw), -1.0, np.float16)
    lidv = (dst & 127).astype(np.float16)
    for blk in range(NBLK):
        m = g == blk
        cap = caps[blk]
        flat = np.zeros(NCORES * NB * B * cap, np.int64)
        addr = ((c[m] * NB + b[m]) * B + r[m]) * cap + j[m]
        flat[addr] = src[m] - blk * BLOCK         # in-block idx (< 32768)
        # [n] slot stream -> [16, n/16]: slot i -> (i%16, i//16)
        idxs.append(flat.reshape(NCORES, NB, (B * cap) // 16, 16)
                    .transpose(0, 1, 3, 2).astype(np.int16))
        pos = r[m] * cap + j[m]                   # slot within batch stream
        laddr = (((c[m] * NB + b[m]) * 128 + (pos & 127)) * (B * tw)
                 + B * btb[blk] + (pos >> 7))
        lids_flat[laddr] = lidv[m]
    # [NCORES, NB, 16, Wtot] single packed idx tensor (block-major cols)
    idxcat = np.concatenate(idxs, axis=3)
    # [NCORES, 128, NB*B*tw] lid plane, SBUF-resident on device
    lids = (lids_flat.reshape(NCORES, NB, 128, B * tw)
            .transpose(0, 2, 1, 3).reshape(NCORES, 128, NB * B * tw))

    cfg = {"caps": tuple(caps), "tw": int(tw), "btb": tuple(btb)}
    data = {"idxcat": idxcat, "lids": lids,
            "degT": deg.reshape(NW, 128).T.copy()}
    return cfg, data


def _win_tiles(cfg, r):
    """Tile indices (within a batch's tile grid) owned by window r."""
    caps, btb = cfg["caps"], cfg["btb"]
    tiles = []
    for blk in range(NBLK):
        cb = caps[blk] // 128
        base = B * btb[blk] + r * cb
        tiles.extend(range(base, base + cb))
    return tiles


def _build_nc(cfg):
    from concourse import bacc, bass, mybir
    import concourse.tile as tile
    from concourse import library_config
    import contextlib

    dt = mybir.dt
    caps, tw, btb = cfg["caps"], cfg["tw"], cfg["btb"]
    bases = [0, BLOCK, 2 * BLOCK, 3 * BLOCK]
    sizes = [BLOCK, BLOCK, BLOCK, NPAD - 3 * BLOCK]

    # consts column layout: iota | ident | W1 | W2 | bt1 | bt2 | degw
    OFF_IOTA, OFF_ID, OFF_W1, OFF_W2 = 0, 128, 256, 384
    OFF_B1, OFF_B2, OFF_DEGW = 512, 640, 768
    CC = 768 + WPC
    wcols = [(B * caps[blk]) // 16 for blk in range(NBLK)]
    woff = [0]
    for wc in wcols:
        woff.append(woff[-1] + wc)

    nc = bacc.Bacc("TRN2", target_bir_lowering=False, debug=False,
                   num_devices=NCORES)
    xs16 = nc.dram_tensor("xs16", [SH, F], dt.float16, kind="ExternalInput")
    consts = nc.dram_tensor("consts", [128, CC], dt.float32,
                            kind="ExternalInput")
    idxcat = nc.dram_tensor("idxcat", [NB, 16, woff[-1]], dt.int16,
                            kind="ExternalInput")
    lidt = nc.dram_tensor("lids", [128, NB * B * tw], dt.float16,
                          kind="ExternalInput")
    idxr = [
        nc.dram_tensor(f"idxr{blk}", [NB, 128, (B * caps[blk]) // 16],
                       dt.int16, kind="Internal")
        for blk in range(NBLK)
    ]
    shard1 = nc.dram_tensor("shard1", [SH, F], dt.float32, kind="Internal")
    shard2 = nc.dram_tensor("shard2", [SH, F], dt.float32, kind="Internal")
    table = nc.dram_tensor("table", [NPAD, F], dt.float32, kind="Internal",
                           addr_space="Shared")
    out16 = nc.dram_tensor("out16", [SH, F], dt.float16,
                           kind="ExternalOutput")

    groups = [list(range(NCORES))]

    with tile.TileContext(nc) as tc:
        ctx = contextlib.ExitStack()
        with ctx:
            cpool = ctx.enter_context(tc.tile_pool(name="const", bufs=1))
            bpool = ctx.enter_context(tc.tile_pool(name="build", bufs=3))
            mpool = ctx.enter_context(tc.tile_pool(name="msg", bufs=2))
            spool = ctx.enter_context(tc.tile_pool(name="sprep", bufs=6))
            Spool = ctx.enter_context(tc.tile_pool(name="onehot", bufs=2))
            epool = ctx.enter_context(tc.tile_pool(name="epi", bufs=3))
            pps = ctx.enter_context(tc.tile_pool(name="ps", bufs=2, space="PSUM"))

            nc.gpsimd.load_library(library_config.mlp)

            # ---- constants: one DMA, use column slices of the tile
            t_cc = cpool.tile([128, CC], dt.float32, tag="cc")
            nc.sync.dma_start(t_cc[:], consts.ap()[:, :])
            t_iota = t_cc[:, OFF_IOTA : OFF_IOTA + 128]
            t_id = t_cc[:, OFF_ID : OFF_ID + 128]
            t_w1 = t_cc[:, OFF_W1 : OFF_W1 + 128]
            t_w2 = t_cc[:, OFF_W2 : OFF_W2 + 128]
            t_b1 = t_cc[:, OFF_B1 : OFF_B1 + 128]
            t_b2 = t_cc[:, OFF_B2 : OFF_B2 + 128]
            t_dinw = cpool.tile([128, WPC], dt.float32, tag="dinw")
            nc.vector.reciprocal(
                t_dinw[:], t_cc[:, OFF_DEGW : OFF_DEGW + WPC])
            nc.scalar.activation(t_dinw[:], t_dinw[:],
                                 mybir.ActivationFunctionType.Sqrt)
            t_lidall = cpool.tile([128, NB * B * tw], dt.float16, tag="lida")
            nc.sync.dma_start(t_lidall[:], lidt.ap()[:, :])

            # ---- replicate packed gather indices to 128 partitions
            for blk in range(NBLK):
                for kk in range(8):
                    nc.sync.dma_start(
                        idxr[blk].ap()[:, 16 * kk : 16 * kk + 16, :],
                        idxcat.ap()[:, :, woff[blk] : woff[blk + 1]],
                    )

            # ---- layer-1 table shard: shard1 = (dinv * x) @ W1
            def build_win(bt):
                """bt: window index, ScalarValue expr or int."""
                t_x16 = bpool.tile([128, F], dt.float16, tag="x16")
                nc.sync.dma_start(t_x16[:], xs16.ap()[bass.ds(bt * 128, 128), :])
                t_x = bpool.tile([128, F], dt.float32, tag="x")
                nc.vector.tensor_copy(t_x[:], t_x16[:])
                t_xs = bpool.tile([128, F], dt.float32, tag="xs")
                nc.vector.tensor_scalar(
                    t_xs[:], t_x[:], t_dinw[:, bass.ds(bt, 1)], None,
                    mybir.AluOpType.mult,
                )
                p_xT = pps.tile([128, 128], dt.float32, tag="xT")
                nc.tensor.transpose(p_xT[:], t_xs[:], t_id[:])
                t_xsT = bpool.tile([128, F], dt.float32, tag="xsT")
                nc.vector.tensor_copy(t_xsT[:], p_xT[:])
                p_h = pps.tile([128, F], dt.float32, tag="h")
                nc.tensor.matmul(p_h[:], t_xsT[:], t_w1[:], start=True, stop=True)
                t_h = bpool.tile([128, F], dt.float32, tag="h")
                nc.vector.tensor_copy(t_h[:], p_h[:])
                nc.sync.dma_start(shard1.ap()[bass.ds(bt * 128, 128), :], t_h[:])

            with tc.For_i(0, WPC, 2) as bt:
                build_win(bt)
                build_win(bt + 1)

            # ---- publish full layer-1 table
            tc.strict_bb_all_engine_barrier()
            nc.gpsimd.collective_compute(
                "AllGather", mybir.AluOpType.bypass, replica_groups=groups,
                ins=[shard1.ap().opt()], outs=[table.ap().opt()],
            )
            tc.strict_bb_all_engine_barrier()

            def gather_batch(b, last):
                """Process gather batch b (ScalarValue expr or int).

                last=False: epilogue fuses the layer-2 shard build into
                shard2.  last=True: epilogue writes the fp16 output.
                """
                t_bias = t_b2 if last else t_b1
                t_msg = mpool.tile([128, B * tw, F], dt.float32, tag="msg")
                t_lid = spool.tile([128, B * tw], dt.float32, tag="lid")
                nc.vector.tensor_copy(
                    t_lid[:], t_lidall[:, bass.ds(b * (B * tw), B * tw)])
                # all one-hot tiles for the batch in one broadcast is_equal:
                # S[p, t, j] = (lid[p, t] == j)
                t_S = Spool.tile([128, B * tw, 128], dt.float32, tag="S")
                nc.vector.tensor_tensor(
                    t_S[:],
                    t_lid[:].broadcast_to([128, B * tw, 128]),
                    t_iota[:, None, :].broadcast_to([128, B * tw, 128]),
                    mybir.AluOpType.is_equal,
                )
                for blk in range(NBLK):
                    cap = caps[blk]
                    t_ix = spool.tile([128, (B * cap) // 16], dt.int16,
                                      tag=f"ix{blk}")
                    nc.sync.dma_start(t_ix[:], idxr[blk].ap()[b, :, :])
                    t0 = B * btb[blk]
                    nc.gpsimd.dma_gather(
                        t_msg[:, t0 : t0 + (B * cap) // 128, :],
                        table.ap()[bases[blk] : bases[blk] + sizes[blk], :],
                        t_ix[:],
                        B * cap, B * cap, F,
                        single_packet=False,
                    )
                for r in range(B):
                    k = b * B + r              # window index within core
                    p_agg = pps.tile([128, F], dt.float32, tag="agg")
                    wt = _win_tiles(cfg, r)
                    for jj, t in enumerate(wt):
                        nc.tensor.matmul(
                            p_agg[:], t_S[:, t, :], t_msg[:, t, :],
                            start=(jj == 0), stop=(jj == len(wt) - 1),
                        )
                    t_e = epool.tile([128, F], dt.float32, tag="e")
                    nc.vector.tensor_scalar(
                        t_e[:], p_agg[:], t_dinw[:, bass.ds(k, 1)], None,
                        mybir.AluOpType.mult,
                    )
                    nc.vector.tensor_tensor(
                        t_e[:], t_e[:], t_bias[:], mybir.AluOpType.add
                    )
                    t_h = epool.tile([128, F], dt.float32, tag="h")
                    nc.scalar.activation(
                        t_h[:], t_e[:], mybir.ActivationFunctionType.Relu
                    )
                    if last:
                        t_o16 = epool.tile([128, F], dt.float16, tag="o16")
                        nc.vector.tensor_copy(t_o16[:], t_h[:])
                        nc.sync.dma_start(
                            out16.ap()[bass.ds(k * 128, 128), :], t_o16[:]
                        )
                    else:
                        # fused layer-2 shard build: (dinv*h) @ W2
                        t_hs = epool.tile([128, F], dt.float32, tag="hs")
                        nc.vector.tensor_scalar(
                            t_hs[:], t_h[:], t_dinw[:, bass.ds(k, 1)], None,
                            mybir.AluOpType.mult,
                        )
                        p_hT = pps.tile([128, 128], dt.float32, tag="xT")
                        nc.tensor.transpose(p_hT[:], t_hs[:], t_id[:])
                        t_hT = epool.tile([128, F], dt.float32, tag="hT")
                        nc.vector.tensor_copy(t_hT[:], p_hT[:])
                        p_h2 = pps.tile([128, F], dt.float32, tag="h")
                        nc.tensor.matmul(p_h2[:], t_hT[:], t_w2[:],
                                         start=True, stop=True)
                        t_h2 = epool.tile([128, F], dt.float32, tag="h2")
                        nc.vector.tensor_copy(t_h2[:], p_h2[:])
                        nc.sync.dma_start(
                            shard2.ap()[bass.ds(k * 128, 128), :], t_h2[:]
                        )

            def gather_layer(last):
                # NB = 49: unrolled-by-2 hardware loop over 48 + static tail
                with tc.For_i(0, NB - 1, 2) as b:
                    gather_batch(b, last)
                    gather_batch(b + 1, last)
                gather_batch(NB - 1, last)

            gather_layer(last=False)

            # ---- publish full layer-2 table (reuses `table`)
            tc.strict_bb_all_engine_barrier()
            nc.gpsimd.collective_compute(
                "AllGather", mybir.AluOpType.bypass, replica_groups=groups,
                ins=[shard2.ap().opt()], outs=[table.ap().opt()],
            )
            tc.strict_bb_all_engine_barrier()

            gather_layer(last=True)

    nc.compile()
    return nc


def _aot_compile(nc):
    """AOT-compile the 8-core SPMD executable (no data, no device calls
    beyond compilation). Returns everything needed to run it."""
    from concourse import bass2jax, mybir
    import jax
    import jax.numpy as jnp
    from jax.sharding import Mesh, PartitionSpec, NamedSharding
    from jax.experimental.shard_map import shard_map

    bass2jax.install_neuronx_cc_hook()
    partition_name = (nc.partition_id_tensor.name
                      if nc.partition_id_tensor else None)
    in_names, out_names, out_avals = [], [], []
    for alloc in nc.m.functions[0].allocations:
        if not isinstance(alloc, mybir.MemoryLocationSet):
            continue
        name = alloc.memorylocations[0].name
        if alloc.kind == "ExternalInput":
            if name != partition_name:
                in_names.append(name)
        elif alloc.kind == "ExternalOutput":
            out_names.append(name)
            out_avals.append(jax.core.ShapedArray(
                tuple(alloc.tensor_shape), mybir.dt.np(alloc.dtype)))
    n_params = len(in_names)
    n_outs = len(out_avals)
    in_names_all = (in_names + out_names
                    + ([partition_name] if partition_name else []))

    def _body(*args):
        operands = list(args)
        if partition_name is not None:
            operands.append(bass2jax.partition_id_tensor())
        outs = bass2jax._bass_exec_p.bind(
            *operands, out_avals=tuple(out_avals),
            in_names=tuple(in_names_all), out_names=tuple(out_names),
            lowering_input_output_aliases=(), sim_require_finite=True,
            sim_require_nnan=True, nc=nc)
        return tuple(outs)

    devices = jax.devices()[:NCORES]
    mesh = Mesh(np.asarray(devices), ("core",))
    spec = NamedSharding(mesh, PartitionSpec("core"))
    in_specs = (PartitionSpec("core"),) * (n_params + n_outs)
    out_specs = (PartitionSpec("core"),) * n_outs
    donate = tuple(range(n_params, n_params + n_outs))
    sharded = jax.jit(shard_map(_body, mesh=mesh, in_specs=in_specs,
                                out_specs=out_specs, check_rep=False),
                      donate_argnums=donate, keep_unused=True)

    # NOTE: per-core BIR shapes concat along axis 0 across the 8 cores
    def _glob(aval):
        return jax.ShapeDtypeStruct(
            (NCORES * aval.shape[0], *aval.shape[1:]), aval.dtype)

    in_structs = []   # filled by caller lookup via in_names order
    self_shapes = {}
    for alloc in nc.m.functions[0].allocations:
        if not isinstance(alloc, mybir.MemoryLocationSet):
            continue
        name = alloc.memorylocations[0].name
        if alloc.kind == "ExternalInput" and name != partition_name:
            self_shapes[name] = (tuple(alloc.tensor_shape),
                                 mybir.dt.np(alloc.dtype))
    for name in in_names:
        shape, dtype = self_shapes[name]
        in_structs.append(jax.ShapeDtypeStruct(
            (NCORES * shape[0], *shape[1:]), dtype))
    out_structs = [_glob(a) for a in out_avals]

    compiled = sharded.lower(*in_structs, *out_structs).compile()

    zero_fns = []
    for s in out_structs:
        zero_fns.append(
            jax.jit(lambda s=s: jnp.zeros(s.shape, s.dtype),
                    out_shardings=spec).lower().compile())

    return {"compiled": compiled, "zero_fns": zero_fns,
            "in_names": in_names, "out_names": out_names,
            "out_avals": out_avals, "spec": spec}


def kernel(x, edge_index, W1, b1, W2, b2):
    global _compiled, _last_wall_s
    import jax

    x = np.asarray(x, np.float32)
    edge_index = np.asarray(edge_index)
    cfg, data = _host_prep(edge_index)
    if _compiled is None or _compiled[1] != cfg:
        nc = _build_nc(cfg)
        _compiled = ((nc, _aot_compile(nc)), cfg)
    nc, rt = _compiled[0]

    xpad = np.zeros((NPAD, F), np.float16)
    xpad[:N] = x.astype(np.float16)
    iota = np.tile(np.arange(128, dtype=np.float32)[None, :], (128, 1))
    ident = np.eye(128, dtype=np.float32)
    bt1 = np.tile(np.asarray(b1, np.float32)[None, :], (128, 1))
    bt2 = np.tile(np.asarray(b2, np.float32)[None, :], (128, 1))
    w1 = np.asarray(W1, np.float32)
    w2 = np.asarray(W2, np.float32)
    in_maps = []
    for c in range(NCORES):
        consts = np.concatenate(
            [iota, ident, w1, w2, bt1, bt2,
             data["degT"][:, c * WPC : (c + 1) * WPC]], axis=1)
        in_maps.append({
            "xs16": xpad[c * SH : (c + 1) * SH],
            "consts": np.ascontiguousarray(consts),
            "idxcat": data["idxcat"][c],
            "lids": data["lids"][c],
        })

    # ---- timed device window: h2d + execute + d2h
    t0 = time.time()
    concat_in = [
        np.concatenate([np.asarray(in_maps[c][name]) for c in range(NCORES)],
                       axis=0)
        for name in rt["in_names"]
    ]
    shardings = rt["compiled"].input_shardings[0]
    dev_in = [jax.device_put(a, s) for a, s in zip(concat_in, shardings)]
    dev_zero = [zf() for zf in rt["zero_fns"]]
    out_arrs = rt["compiled"](*dev_in, *dev_zero)
    host_out = [np.asarray(o) for o in out_arrs]
    _last_wall_s = time.time() - t0

    out = host_out[rt["out_names"].index("out16")]
    return out[:N].astype(np.float32)
